# revision 6
# baseline (speedup 1.0000x reference)
"""Enformer relative-position attention on 8 Trainium2 NeuronCores.

Sharding: core c handles batch b = c//2 and head group g = c%2 (4 heads).
Each core computes its head-group's slice of the attention and a partial
output projection (contraction over its 768 value channels); the host sums
the two partials per batch and adds b_out.

relative_shift is implemented with a DRAM stride trick: the per-row-chunk
rel-logit band [128, 1664] is written to scratch DRAM with row stride 1664
and read back with row stride 1663 at offset 127, which realizes
out[p, j] = band[p, j - p + 127] exactly. The shifted band is added into
the content-logit PSUM with identity matmuls, so softmax reads fused
content+rel logits straight out of PSUM.

v4 layout:
- the host pre-casts all big inputs to f16, pre-transposes x, and
  pre-rearranges the weight matrices into SBUF layout, so every load is a
  contiguous castless DMA (large descriptors, no xbar transposes on the
  critical path).
- band stages interleave into the projection phase; band scratch writes
  ride the sync HWDGE ring while shifted-band reads ride the gpsimd queue
  so the two never serialize against each other.
- softmax and the output projection run on [128,512] single-bank PSUM
  chunks from one 8-buffer pool (exp per chunk + 3-part row-sum), which
  keeps ~8 units in flight instead of 2.
- the 64-partition QK/band matmuls are emitted in row-group pairs (auto
  tile_position 0/64) so the PE runs two at once; the identity band-adds
  are split into (0,0)/(64,64) diagonal halves.
"""
import math
import sys
from contextlib import ExitStack

import numpy as np

for _p in ("/opt/trn_rl_repo",):
    if _p not in sys.path:
        sys.path.append(_p)

import concourse.bass as bass
import concourse.mybir as mybir
import concourse.tile as tile
from concourse.bass_utils import run_bass_kernel_spmd
from concourse.masks import make_identity

F32 = mybir.dt.float32
F16 = mybir.dt.float16

B, N, D = 4, 1536, 1536
H, DK, DV, NRPF = 8, 64, 192, 192
HG = 4                  # heads per core
F = HG * DK             # 256 q/k columns per core
DVG = HG * DV           # 768 v columns per core
R = 2 * N - 1           # 3071 relative positions
RP = 3072               # padded
BW = 1664               # band window width per 128-row chunk (1663 + 1 pad)
NT = N // 128           # 12 row chunks
ACT_F = mybir.ActivationFunctionType


# ---------------------------------------------------------------- host math
def _positions_np():
    """get_positional_embed(1536, 192) from the Enformer reference, in numpy."""
    seq_len, feature_size = N, NRPF
    distances = np.arange(-seq_len + 1, seq_len, dtype=np.float64)
    absd = np.abs(distances)[:, None]
    nb = feature_size // 6
    max_range = math.log(seq_len) / math.log(2.0)
    half_life = 2.0 ** np.linspace(3.0, max_range, nb)
    pe_exp = np.exp(-math.log(2.0) / half_life[None, :] * absd)
    center_widths = 2.0 ** np.arange(1, nb + 1, dtype=np.float64) - 1.0
    pe_cm = (center_widths[None, :] > absd).astype(np.float64)
    stddev = seq_len / (2.0 * nb)
    start_mean = seq_len / nb
    mean = np.linspace(start_mean, float(seq_len), nb)[None, :]
    conc = (mean / stddev) ** 2
    rate = mean / (stddev ** 2)
    with np.errstate(divide="ignore"):
        log_unnorm = (conc - 1.0) * np.where(absd > 0, np.log(np.maximum(absd, 1e-300)), -np.inf)
    log_unnorm = np.where(absd > 0, log_unnorm, np.where(conc - 1.0 > 0, -np.inf, 0.0)) - rate * absd
    lgam = np.vectorize(math.lgamma)(conc)
    log_norm = lgam - conc * np.log(rate)
    probs = np.exp(log_unnorm - log_norm) + 1e-8
    pe_g = probs / probs.max(axis=-1, keepdims=True)
    emb = np.concatenate([pe_exp, pe_cm, pe_g], axis=-1)
    full = np.concatenate([emb, np.sign(distances)[:, None] * emb], axis=-1)
    return full.astype(np.float32)  # (3071, 192)


def _sbuf_weight_layout(w):
    """[D, C] -> [128, NT*C]: partition = d%128, free = (d//128, c)."""
    d, c = w.shape
    return np.ascontiguousarray(
        w.reshape(d // 128, 128, c).transpose(1, 0, 2).reshape(128, -1))


# ---------------------------------------------------------------- device IR
def _build_nc():
    nc = bass.Bass()
    xT = nc.declare_dram_parameter("xT", [D, N], F16, isOutput=False)
    wqr = nc.declare_dram_parameter("wqr", [128, NT * F], F16, isOutput=False)
    wkr = nc.declare_dram_parameter("wkr", [128, NT * F], F16, isOutput=False)
    wvr = nc.declare_dram_parameter("wvr", [128, NT * DVG], F16, isOutput=False)
    wrel = nc.declare_dram_parameter("wrel", [NRPF, F], F16, isOutput=False)
    rcb = nc.declare_dram_parameter("rcb", [128, 2], F32, isOutput=False)
    rpb = nc.declare_dram_parameter("rpb", [128, 2], F32, isOutput=False)
    wout = nc.declare_dram_parameter("wout", [DVG, D], F16, isOutput=False)
    post = nc.declare_dram_parameter("post", [NRPF, RP], F16, isOutput=False)
    out = nc.declare_dram_parameter("out", [N, D], F16, isOutput=True)

    with tile.TileContext(nc) as tc, ExitStack() as ctx:
        sing = ctx.enter_context(tc.tile_pool(name="sing", bufs=1))
        dscr = ctx.enter_context(tc.tile_pool(name="dscr", bufs=8, space="DRAM"))

        # ---- persistent constants
        ident16 = sing.tile([128, 128], F16, tag="ident")
        make_identity(nc, ident16[:, :])
        rcb32 = sing.tile([128, 2], F32, tag="rcb")
        rpb32 = sing.tile([128, 2], F32, tag="rpb")
        wout16 = [sing.tile([128, D], F16, tag=f"wo{dvt}", name=f"wout16_{dvt}")
                  for dvt in range(6)]
        relkT = [sing.tile([128, RP], F16, tag=f"relkT{ft}", name=f"relkT_{ft}")
                 for ft in range(2)]
        qcT = [sing.tile([128, N], F16, tag=f"qcT{t}", name=f"qcT_{t}") for t in range(2)]
        qbT = [sing.tile([128, N], F16, tag=f"qbT{t}", name=f"qbT_{t}") for t in range(2)]
        kT = [sing.tile([128, N], F16, tag=f"kT{t}", name=f"kT_{t}") for t in range(2)]
        v16 = [sing.tile([128, DVG], F16, tag=f"v{c}", name=f"v16_{c}") for c in range(NT)]
        outtT = [sing.tile([128, 512], F16, tag=f"outt{k}", name=f"outtT_{k}")
                 for k in range(6)]
        scrs = [dscr.tile([128 * BW], F16, tag=f"scr{i}", name=f"scr_{i}")
                for i in range(48)]

        # ========== phase A: projections + rel_k + band stages =============
        with tc.tile_pool(name="pa", bufs=1) as pa, \
             tc.tile_pool(name="psA", bufs=8, space="PSUM") as ps_small:
            # x window loads ride the sync ring, weights the scalar ring;
            # everything is contiguous f16, so startup is bandwidth-bound
            # only on what window 0 actually needs.
            xtsl = []
            for w in range(3):
                xts = pa.tile([128, NT, 4, 128], F16, tag="xts", bufs=3,
                              name=f"xts_{w}")
                xtsl.append(xts)
                nc.sync.dma_start(
                    out=xts,
                    in_=bass.AP(tensor=xT[:, :].tensor, offset=w * 512,
                                ap=[[N, 128], [128 * N, NT], [1, 512]]))
            wq16b = pa.tile([128, NT, F], F16, tag="wqb", name="wq16b")
            nc.scalar.dma_start(out=wq16b, in_=wqr[:, :])
            wk16b = pa.tile([128, NT, F], F16, tag="wkb", name="wk16b")
            nc.scalar.dma_start(out=wk16b, in_=wkr[:, :])
            nc.sync.dma_start(out=rcb32, in_=rcb[:, :])
            nc.sync.dma_start(out=rpb32, in_=rpb[:, :])
            wv16b = pa.tile([128, NT, DVG], F16, tag="wvb", name="wv16b")
            nc.scalar.dma_start(out=wv16b, in_=wvr[:, :])
            wrel16a = pa.tile([128, F], F16, tag="wrela")
            wrel16b = pa.tile([64, F], F16, tag="wrelb")
            nc.scalar.dma_start(out=wrel16a, in_=wrel[0:128, :])
            nc.scalar.dma_start(out=wrel16b, in_=wrel[128:192, :])
            post16a = pa.tile([128, RP], F16, tag="posta")
            post16b = pa.tile([64, RP], F16, tag="postb")
            nc.scalar.dma_start(out=post16a, in_=post[0:128, :])
            nc.scalar.dma_start(out=post16b, in_=post[128:192, :])
            wq16 = [wq16b[:, dt, :] for dt in range(NT)]
            wk16 = [wk16b[:, dt, :] for dt in range(NT)]
            wv16 = [wv16b[:, dt, :] for dt in range(NT)]

            def emit_window(w):
                xts = xtsl[w]
                for ft in range(2):
                    pq = ps_small.tile([128, 512], F32, tag="small", name="pq")
                    for dt in range(NT):
                        nc.tensor.matmul(pq, wq16[dt][:, ft * 128:(ft + 1) * 128],
                                         xts[:, dt, :, :].rearrange("p a b -> p (a b)"),
                                         start=(dt == 0), stop=(dt == NT - 1))
                    nc.vector.tensor_scalar(out=qcT[ft][:, w * 512:(w + 1) * 512], in0=pq,
                                            scalar1=0.125, scalar2=rcb32[:, ft:ft + 1],
                                            op0=mybir.AluOpType.mult, op1=mybir.AluOpType.add)
                    nc.vector.tensor_scalar(out=qbT[ft][:, w * 512:(w + 1) * 512], in0=pq,
                                            scalar1=0.125, scalar2=rpb32[:, ft:ft + 1],
                                            op0=mybir.AluOpType.mult, op1=mybir.AluOpType.add)
                    pk = ps_small.tile([128, 512], F32, tag="small", name="pk")
                    for dt in range(NT):
                        nc.tensor.matmul(pk, wk16[dt][:, ft * 128:(ft + 1) * 128],
                                         xts[:, dt, :, :].rearrange("p a b -> p (a b)"),
                                         start=(dt == 0), stop=(dt == NT - 1))
                    nc.scalar.copy(kT[ft][:, w * 512:(w + 1) * 512], pk)
                for c in range(4):
                    pva = ps_small.tile([128, 512], F32, tag="small", name="pva")
                    pvb = ps_small.tile([128, 512], F32, tag="small", name="pvb")
                    for dt in range(NT):
                        nc.tensor.matmul(pva, xts[:, dt, c, :],
                                         wv16[dt][:, 0:512], start=(dt == 0), stop=(dt == NT - 1))
                        nc.tensor.matmul(pvb[:, 0:256], xts[:, dt, c, :],
                                         wv16[dt][:, 512:768], start=(dt == 0), stop=(dt == NT - 1))
                    nc.vector.tensor_copy(v16[w * 4 + c][:, 0:512], pva)
                    nc.scalar.copy(v16[w * 4 + c][:, 512:768], pvb[:, 0:256])

            def band_pair(s, c):
                # the two heads of a pair occupy PE row groups 0:64 / 64:128;
                # interleaving their matmuls lets the PE run them two at a time
                iw, p = divmod(s, 2)
                I = iw * 4 + c
                r0 = 1408 - 128 * I
                bands, psbs = [], []
                for hh in range(2):
                    bands.append(pa.tile([128, BW], F16, tag="band16", bufs=4,
                                         name=f"band16_{hh}"))
                for k in range(4):
                    wdt = 512 if k < 3 else 128
                    for hh in range(2):
                        h = 2 * p + hh
                        ft, base = h // 2, (h % 2) * 64
                        psb = ps_small.tile([128, 512], F32, tag="small", name="psb")
                        psbs.append(psb)
                        nc.tensor.matmul(
                            psb[:, 0:wdt],
                            qbT[ft][base:base + 64, I * 128:(I + 1) * 128],
                            relkT[ft][base:base + 64, r0 + k * 512:r0 + k * 512 + wdt],
                            start=True, stop=True)
                for k in range(4):
                    wdt = 512 if k < 3 else 128
                    for hh in range(2):
                        psb = psbs[k * 2 + hh]
                        if (k + hh) % 2 == 0:
                            nc.vector.tensor_copy(bands[hh][:, k * 512:k * 512 + wdt],
                                                  psb[:, 0:wdt])
                        else:
                            nc.scalar.copy(bands[hh][:, k * 512:k * 512 + wdt],
                                           psb[:, 0:wdt])
                for hh in range(2):
                    it = s * 8 + hh * 4 + c
                    nc.sync.dma_start(
                        out=bass.AP(tensor=scrs[it][:].tensor, offset=0,
                                    ap=[[BW, 128], [1, BW]]),
                        in_=bands[hh])

            def emit_band(s):
                for c in range(4):
                    band_pair(s, c)

            emit_window(0)
            # rel_k after the first window so startup isn't load-bound
            for ft in range(2):
                for rw in range(6):
                    pr = ps_small.tile([128, 512], F32, tag="small", name="pr")
                    nc.tensor.matmul(pr, wrel16a[:, ft * 128:(ft + 1) * 128],
                                     post16a[:, rw * 512:(rw + 1) * 512],
                                     start=True, stop=False)
                    nc.tensor.matmul(pr, wrel16b[:, ft * 128:(ft + 1) * 128],
                                     post16b[:, rw * 512:(rw + 1) * 512],
                                     start=False, stop=True)
                    nc.vector.tensor_copy(relkT[ft][:, rw * 512:(rw + 1) * 512], pr)
            emit_window(1)
            emit_band(0)
            emit_band(1)
            emit_window(2)
            emit_band(2)
            emit_band(3)
            emit_band(4)
            emit_band(5)

        # ================= phases B/C/D: attention (scoped SBUF) ===========
        ps8 = ctx.enter_context(tc.tile_pool(name="ps8", bufs=8, space="PSUM"))
        with tc.tile_pool(name="pb", bufs=1) as pb:
            for dvt in range(6):
                nc.gpsimd.dma_start(out=wout16[dvt],
                                    in_=wout[dvt * 128:(dvt + 1) * 128, :])
            at2 = [[pb.tile([128, NT, 4, 128], F16, tag=f"ats{g}{hh}", name=f"ats_{g}_{hh}")
                    for hh in range(2)] for g in range(2)]

            def soft_pair(s, c):
                iw, p = divmod(s, 2)
                g = s % 2
                I = iw * 4 + c
                ft = p
                bss, a32s, sps = [], [], []
                for hh in range(2):
                    it = s * 8 + hh * 4 + c
                    bs16 = pb.tile([128, N], F16, tag="bs16", bufs=4,
                                   name=f"bs16_{hh}")
                    bss.append(bs16)
                    nc.gpsimd.dma_start(
                        out=bs16,
                        in_=bass.AP(tensor=scrs[it][:].tensor, offset=127,
                                    ap=[[BW - 1, 128], [1, N]]))
                    a32s.append(pb.tile([128, N], F32, tag="a32", bufs=3,
                                        name=f"a32_{hh}"))
                    sps.append(pb.tile([128, 4], F32, tag="s32", bufs=4,
                                       name=f"s32_{hh}"))
                # per 512-chunk: paired content matmuls (row groups 0/64),
                # diagonal-half identity band adds, then exp straight off the
                # single-bank PSUM chunk — keeps many chunks in flight
                for jw in range(3):
                    pscs = []
                    for hh in range(2):
                        base = hh * 64
                        psc = ps8.tile([128, 512], F32, tag="ps", name="psc")
                        pscs.append(psc)
                        nc.tensor.matmul(
                            psc,
                            qcT[ft][base:base + 64, I * 128:(I + 1) * 128],
                            kT[ft][base:base + 64, jw * 512:(jw + 1) * 512],
                            start=True, stop=False)
                    for hh in range(2):
                        nc.tensor.matmul(
                            pscs[hh][0:64, :],
                            ident16[0:64, 0:64],
                            bss[hh][0:64, jw * 512:(jw + 1) * 512],
                            start=False, stop=False)
                        nc.tensor.matmul(
                            pscs[hh][64:128, :],
                            ident16[64:128, 64:128],
                            bss[hh][64:128, jw * 512:(jw + 1) * 512],
                            start=False, stop=True)
                    for hh in range(2):
                        nc.scalar.activation(
                            out=a32s[hh][:, jw * 512:(jw + 1) * 512],
                            in_=pscs[hh], func=ACT_F.Exp,
                            accum_out=sps[hh][:, jw:jw + 1])
                for hh in range(2):
                    nc.vector.tensor_add(sps[hh][:, 3:4], sps[hh][:, 0:1],
                                         sps[hh][:, 1:2])
                    nc.vector.tensor_add(sps[hh][:, 3:4], sps[hh][:, 3:4],
                                         sps[hh][:, 2:3])
                    rs32 = pb.tile([128, 1], F32, tag="rs32", bufs=4, name="rs32")
                    nc.vector.reciprocal(rs32, sps[hh][:, 3:4])
                    a16n = pb.tile([128, N], F16, tag="a16n", bufs=3, name="a16n")
                    nc.vector.tensor_scalar_mul(a16n, a32s[hh], rs32)
                    nc.sync.dma_start_transpose(at2[g][hh][:, :, c, :], a16n)

            def emit_B(s):
                for c in range(4):
                    soft_pair(s, c)

            def emit_PV(s):
                iw, p = divmod(s, 2)
                g = s % 2
                h0c, h1c = (2 * p) * DV, (2 * p + 1) * DV
                for k in range(3):
                    pspv = ps8.tile([128, 512], F32, tag="ps", name="pspv")
                    for jt in range(NT):
                        st, sp = jt == 0, jt == NT - 1
                        at0 = at2[g][0][:, jt, :, :].rearrange("p a b -> p (a b)")
                        at1 = at2[g][1][:, jt, :, :].rearrange("p a b -> p (a b)")
                        if k == 0:
                            nc.tensor.matmul(pspv, v16[jt][:, h0c:h0c + 128],
                                             at0, start=st, stop=sp)
                        elif k == 2:
                            nc.tensor.matmul(pspv, v16[jt][:, h1c + 64:h1c + 192],
                                             at1, start=st, stop=sp)
                        else:
                            nc.tensor.matmul(pspv[0:64, :], v16[jt][:, h0c + 128:h0c + 192],
                                             at0, start=st, stop=sp, tile_position=(0, 0))
                            nc.tensor.matmul(pspv[64:128, :], v16[jt][:, h1c:h1c + 64],
                                             at1, start=st, stop=sp, tile_position=(0, 64))
                    if k % 2 == 0:
                        nc.vector.tensor_copy(outtT[p * 3 + k], pspv)
                    else:
                        nc.scalar.copy(outtT[p * 3 + k], pspv)

            def emit_OUT(iw):
                for c in range(4):
                    I = iw * 4 + c
                    of = pb.tile([128, N], F16, tag="of", bufs=3, name="of")
                    for jw in range(3):
                        pso = ps8.tile([128, 512], F32, tag="ps", name="pso")
                        for dvt in range(6):
                            nc.tensor.matmul(pso,
                                             outtT[dvt][:, c * 128:(c + 1) * 128],
                                             wout16[dvt][:, jw * 512:(jw + 1) * 512],
                                             start=(dvt == 0), stop=(dvt == 5))
                        if (c + jw) % 2 == 0:
                            nc.vector.tensor_copy(of[:, jw * 512:(jw + 1) * 512], pso)
                        else:
                            nc.scalar.copy(of[:, jw * 512:(jw + 1) * 512], pso)
                    nc.gpsimd.dma_start(out=out[I * 128:(I + 1) * 128, :], in_=of)

            # softmax + PV + OUT, software-pipelined across sections
            emit_B(0)
            emit_B(1)
            emit_PV(0)
            emit_B(2)
            emit_PV(1)
            emit_OUT(0)
            emit_B(3)
            emit_PV(2)
            emit_B(4)
            emit_PV(3)
            emit_OUT(1)
            emit_B(5)
            emit_PV(4)
            emit_PV(5)
            emit_OUT(2)
    return nc


# --------------------------------------------------- multi-wait legalization
_legal_counter = [0]


def _legalize_multi_waits(nc, max_waits=1, max_updates=1):
    """Split multi-wait/update instructions into EventSemaphore chains.

    The TRN2 instruction encoding holds one sync-wait and one sync-update
    command; Tile attaches as many as the dependence structure needs, so we
    hoist the extras onto standalone EventSemaphore instructions."""
    for f in nc.m.functions:
        for blk in f.blocks:
            outl, changed = [], False
            for inst in blk.instructions:
                si = inst.sync_info
                if si is not None and si.on_wait and len(si.on_wait) > max_waits:
                    waits = list(si.on_wait)
                    for wcmd in waits[:-max_waits]:
                        ev = mybir.InstEventSemaphore(
                            name=f"legalw-{_legal_counter[0]}", ins=[], outs=[])
                        _legal_counter[0] += 1
                        ev.engine = inst.engine
                        ev.sync_info = mybir.SyncInfo(on_wait=[wcmd], on_update=[])
                        outl.append(ev)
                        changed = True
                    inst.sync_info = mybir.SyncInfo(
                        on_wait=waits[-max_waits:], on_update=list(si.on_update or []))
                    si = inst.sync_info
                if si is not None and si.on_update and len(si.on_update) > max_updates:
                    ups = list(si.on_update)
                    inst.sync_info = mybir.SyncInfo(
                        on_wait=list(si.on_wait or []), on_update=ups[:max_updates])
                    outl.append(inst)
                    for ucmd in ups[max_updates:]:
                        ev = mybir.InstEventSemaphore(
                            name=f"legalu-{_legal_counter[0]}", ins=[], outs=[])
                        _legal_counter[0] += 1
                        ev.engine = inst.engine
                        ev.sync_info = mybir.SyncInfo(on_wait=[], on_update=[ucmd])
                        outl.append(ev)
                    changed = True
                    continue
                outl.append(inst)
            if changed:
                blk.instructions = outl
    return nc


# ------------------------------------------------------------------- driver
_NC_CACHE = {}
LAST = {}


def _get_nc():
    if "nc" not in _NC_CACHE:
        nc = _build_nc()
        _legalize_multi_waits(nc)
        _NC_CACHE["nc"] = nc
    return _NC_CACHE["nc"]


def kernel(x, Wq, Wk, Wv, Wrel, rel_content_bias, rel_pos_bias, Wout, b_out):
    x16 = np.asarray(x, dtype=np.float16)
    Wq16 = np.asarray(Wq, dtype=np.float16)
    Wk16 = np.asarray(Wk, dtype=np.float16)
    Wv16 = np.asarray(Wv, dtype=np.float16)
    Wrel16 = np.asarray(Wrel, dtype=np.float16)
    rcb = np.asarray(rel_content_bias, dtype=np.float32).reshape(H, DK)
    rpb = np.asarray(rel_pos_bias, dtype=np.float32).reshape(H, DK)
    Wout16 = np.asarray(Wout, dtype=np.float16)
    b_out = np.asarray(b_out, dtype=np.float32)

    post = np.zeros((NRPF, RP), dtype=np.float16)
    post[:, :R] = _positions_np().T.astype(np.float16)

    in_maps = []
    for core in range(8):
        b, g = core // 2, core % 2
        f0, v0 = g * F, g * DVG
        rcb_g = np.zeros((128, 2), dtype=np.float32)
        rpb_g = np.zeros((128, 2), dtype=np.float32)
        for t in range(2):
            rcb_g[:, t] = rcb[g * HG + 2 * t: g * HG + 2 * t + 2].reshape(128)
            rpb_g[:, t] = rpb[g * HG + 2 * t: g * HG + 2 * t + 2].reshape(128)
        in_maps.append({
            "xT": np.ascontiguousarray(x16[b].T),
            "wqr": _sbuf_weight_layout(Wq16[:, f0:f0 + F]),
            "wkr": _sbuf_weight_layout(Wk16[:, f0:f0 + F]),
            "wvr": _sbuf_weight_layout(Wv16[:, v0:v0 + DVG]),
            "wrel": np.ascontiguousarray(Wrel16[:, f0:f0 + F]),
            "rcb": rcb_g,
            "rpb": rpb_g,
            "wout": np.ascontiguousarray(Wout16[v0:v0 + DVG, :]),
            "post": post,
        })

    nc = _get_nc()
    res = None
    for attempt in range(3):
        try:
            res = run_bass_kernel_spmd(nc, in_maps, list(range(8)))
            break
        except Exception:
            if attempt == 2:
                raise
    LAST["res"] = res
    parts = [np.asarray(res.results[c]["out"], dtype=np.float32) for c in range(8)]
    out = np.empty((B, N, D), dtype=np.float32)
    for b in range(B):
        out[b] = parts[2 * b] + parts[2 * b + 1] + b_out[None, :]
    return out
